# revision 23
# baseline (speedup 1.0000x reference)
"""Causal self-attention (GPT-2 style) on 8 TRN2 NeuronCores.

Sharding: B=2 x H=12 -> 24 (batch, head) pairs; core c handles batch c//4
and heads [3*(c%4), 3*(c%4)+3). Each core computes QKV for its 3 heads,
causal attention (flash-style, scores^T layout), and a partial output
projection; the host sums the 4 per-batch partials and adds b_proj.

v2: single software-pipelined schedule. Attention for q-block J overlaps
with QKV for block J+1, the output projection for block J-1 and the
softmax finalize for J-1 (side-work units drained from a deque, one per
attention group). The causal diagonal mask is applied as a -30 additive
matmul into PSUM before exp (no DVE masking on the exp->AV critical
path). Softmax normalization avoids ACT entirely: denominator row is
drained with O^T to SBUF, reciprocal via DVE approx, broadcast across
partitions via gpsimd, multiply on DVE. ACT does exp only, which is the
global bottleneck engine; PE is kept dense (p-state!) by the side-work
interleave.

Self-contained: builds the Bass program on first call, runs via
run_bass_kernel_spmd on cores 0-7.
"""
import numpy as np
import ml_dtypes

import concourse.bass as bass
import concourse.mybir as mybir
import concourse.tile as tile
from concourse.bass import ts
from concourse.vector_clock import ScopedClock
from concourse.bass_utils import run_bass_kernel_spmd

# ---------------------------------------------------------------------------
# Workaround for the container's walrus build, which rejects any instruction
# carrying more than ONE sync-wait command ("Too many sync wait commands").
# 1) patch the TileContext tail drain to funnel its wait-set through
#    single-wait NOPs on SP; 2) post-pass that moves excess on_wait entries
#    from any instruction onto single-wait NOPs inserted before it on the
#    same engine (engine stalls on the NOPs, then issues the instruction —
#    semantics preserved).
# ---------------------------------------------------------------------------
_WAIT_LIMIT = 1


def _patched_drain_and_barrier(self, tick_clock, wait_clock):
    nc = self.nc
    carrier = nc.sync.nop()
    wait_clock.add_sem_waits(carrier.ins, ScopedClock({None: tick_clock.global_clock}))
    si = carrier.ins.sync_info
    waits = list(si.on_wait) if si and si.on_wait else []
    if len(waits) > _WAIT_LIMIT:
        si.on_wait = waits[:_WAIT_LIMIT]
        for w in waits[_WAIT_LIMIT:]:
            n2 = nc.sync.nop()
            s2 = n2.ins.sync_info
            if s2 is None:
                n2.ins.sync_info = mybir.SyncInfo(on_wait=[w], on_update=[])
            else:
                s2.on_wait = [w]
    nc.sync.drain()
    nc.all_engine_barrier()
    popped = nc._tile_sem_poison_stack.pop()
    assert popped is self._sem_poison
    nc.clear_and_free_semaphores(list(self.sems.allocated().values()))
    nc.all_engine_barrier()


tile.TileContext._drain_and_barrier = _patched_drain_and_barrier


def _split_multi_waits(nc):
    n_inserted = 0
    for fn in nc.m.functions:
        for blk in fn.blocks:
            new_list = []
            changed = False
            for inst in blk.instructions:
                si = getattr(inst, "sync_info", None)
                waits = list(si.on_wait) if (si is not None and si.on_wait) else []
                if len(waits) > _WAIT_LIMIT:
                    extra = waits[: len(waits) - _WAIT_LIMIT]
                    keep = waits[len(waits) - _WAIT_LIMIT:]
                    for w in extra:
                        nop = mybir.InstNoOp(
                            name=f"wsplit-{n_inserted}",
                            sync_info=mybir.SyncInfo(on_wait=[w], on_update=[]),
                            bass_nofuse=True,
                            engine=inst.engine,
                        )
                        new_list.append(nop)
                        n_inserted += 1
                    si.on_wait = keep
                    changed = True
                new_list.append(inst)
            if changed:
                blk.instructions = new_list
    return n_inserted


# ---------------------------------------------------------------------------
# Problem constants (hardcoded per contract).
# ---------------------------------------------------------------------------
B, S, E, H = 2, 4096, 768, 12
D = 64           # head dim
HPC = 3          # heads per core
EAUG = 832       # 768 + ones/bias row at 768, zero-padded to 6*128+64
NCORES = 8
BF16 = mybir.dt.bfloat16
F32 = mybir.dt.float32
QB = 512         # q-block width (one PSUM bank of fp32)
NQB = S // QB    # 8
NKT = S // 128   # 32 k-tiles

TRACE = False
LAST_EXEC_NS = None

_nc = {}


def _echunks(with_bias):
    # contraction chunks over the (augmented) feature dim
    ch = [(e * 128, 128) for e in range(6)]
    if with_bias:
        ch.append((768, 64))  # ones/bias row (+ zero padding)
    return ch


def _build_program(with_bias):
    nc = bass.Bass()
    xT = nc.dram_tensor("xT", [EAUG, S], BF16, kind="ExternalInput")
    wqk = nc.dram_tensor("wqk", [EAUG, 2 * HPC * D], BF16, kind="ExternalInput")
    wv = nc.dram_tensor("wv", [EAUG, HPC * D], BF16, kind="ExternalInput")
    wp01 = nc.dram_tensor("wp01", [128, E], BF16, kind="ExternalInput")
    wp2 = nc.dram_tensor("wp2", [128, E], BF16, kind="ExternalInput")
    msk = nc.dram_tensor("msk", [128, 128], BF16, kind="ExternalInput")
    idn = nc.dram_tensor("idn", [128, 128], BF16, kind="ExternalInput")
    on1 = nc.dram_tensor("on1", [1, 64], BF16, kind="ExternalInput")
    z64 = nc.dram_tensor("z64", [64, S], BF16, kind="ExternalInput")
    y = nc.dram_tensor("y", [S, E], F32, kind="ExternalOutput")

    ech = _echunks(with_bias)
    NE = len(ech)

    with tile.TileContext(nc) as tc:
        with (
            tc.tile_pool(name="wpool", bufs=1) as wpool,
            tc.tile_pool(name="per", bufs=1) as per,
            tc.tile_pool(name="xch", bufs=12) as xch,
            tc.tile_pool(name="asb", bufs=6) as asb,
            tc.tile_pool(name="osb", bufs=1) as osb,
            tc.tile_pool(name="sps", bufs=2, space="PSUM") as sps,
            tc.tile_pool(name="ops", bufs=2, space="PSUM") as ops,
            tc.tile_pool(name="aux", bufs=2, space="PSUM") as aux,
        ):
            # --- weights + constants to SBUF ---
            wqk_sb, wv_sb = [], []
            for e, (r0, rn) in enumerate(ech):
                t1 = wpool.tile([rn, 2 * HPC * D], BF16, name=f"wqk{e}")
                nc.sync.dma_start(out=t1, in_=wqk[r0:r0 + rn, :])
                wqk_sb.append(t1)
                t2 = wpool.tile([rn, HPC * D], BF16, name=f"wv{e}")
                nc.sync.dma_start(out=t2, in_=wv[r0:r0 + rn, :])
                wv_sb.append(t2)
            wp01_sb = wpool.tile([128, E], BF16, name="wp01_sb")
            nc.sync.dma_start(out=wp01_sb, in_=wp01[:, :])
            wp2_sb = wpool.tile([128, E], BF16, name="wp2_sb")
            nc.sync.dma_start(out=wp2_sb, in_=wp2[:, :])
            msk_sb = wpool.tile([128, 128], BF16, name="msk_sb")
            nc.sync.dma_start(out=msk_sb, in_=msk[:, :])
            idn_sb = wpool.tile([128, 128], BF16, name="idn_sb")
            nc.sync.dma_start(out=idn_sb, in_=idn[:, :])
            on1_sb = wpool.tile([1, 64], BF16, name="on1_sb")
            nc.sync.dma_start(out=on1_sb, in_=on1[:, :])

            # --- persistent intermediates ---
            # Feature-major Q^T/K^T. qt01 packs [q_h0 ; q_h1] on partitions
            # 0:64 / 64:128 (natural f-group drain). qt2 = [q_h2 ; k_h0]
            # (row filler is real data: multiplied by kt zero-pad).
            # kt[h] is per-head with ZEROS in the half where the paired q
            # tile carries the other head: kt0=[k_h0;0], kt1=[0;k_h1],
            # kt2=[k_h2;0]. All matmuls contract over full 128 partitions
            # (K=64 matmuls never un-throttle the PE clock gate).
            qt01 = per.tile([128, S], BF16, name="qt01")
            qt2 = per.tile([128, S], BF16, name="qt2")
            kt0 = per.tile([128, S], BF16, name="kt0")
            kt1 = per.tile([128, S], BF16, name="kt1")
            kt2 = per.tile([128, S], BF16, name="kt2")
            # zero pads arrive via small per-block DMAs inside the QKV
            # units (a single [64, S] DMA is slow enough to gate the first
            # score matmuls)

            def qt_of(h):
                return (qt01, qt01, qt2)[h]

            def kt_of(h):
                return (kt0, kt1, kt2)[h]

            # vtok[h]: token-major V with a ones column per k-tile:
            # cols [65i, 65i+64) = V rows, col 65i+64 = 1.0 (denominator)
            vtok = [per.tile([128, 65 * NKT], BF16, name=f"vtok{h}")
                    for h in range(HPC)]
            for h in range(HPC):
                nc.vector.memset(vtok[h], 1.0)
            # normalized O^T packed for projection: ot01=[o_h0;o_h1],
            # ot2=[o_h2;0]
            ot01 = per.tile([128, S], BF16, name="ot01")
            ot2 = per.tile([128, S], BF16, name="ot2")

            # ----------------------------------------------------------
            # work units
            # ----------------------------------------------------------
            def unit_xdma(tb):
                def run():
                    xc = []
                    for e, (r0, rn) in enumerate(ech):
                        t = xch.tile([rn, QB], BF16, name=f"xc{e}", tag="xc")
                        nc.sync.dma_start(out=t, in_=xT[r0:r0 + rn, ts(tb, QB)])
                        xc.append(t)
                    xc_cur[tb] = xc
                return run

            xc_cur = {}

            qk_done = {}

            def unit_qk(tb, f):
                def run():
                    qk_done[tb] = qk_done.get(tb, 0) + 1
                    xc = xc_cur[tb]
                    qkp = aux.tile([128, QB], F32, name="qkp", tag="aux")
                    for e in range(NE):
                        nc.tensor.matmul(qkp, wqk_sb[e][:, ts(f, 128)], xc[e],
                                         start=(e == 0), stop=(e == NE - 1))
                    if f == 0:      # [q_h0 ; q_h1]
                        nc.vector.tensor_copy(qt01[:, ts(tb, QB)], qkp)
                    elif f == 1:    # [q_h2 ; k_h0]
                        nc.vector.tensor_copy(qt2[:, ts(tb, QB)], qkp)
                        nc.vector.tensor_copy(kt0[0:64, ts(tb, QB)],
                                              qkp[64:128, :])
                        nc.sync.dma_start(out=kt0[64:128, ts(tb, QB)],
                                          in_=z64[:, ts(tb, QB)])
                    else:           # [k_h1 ; k_h2]
                        nc.vector.tensor_copy(kt1[64:128, ts(tb, QB)],
                                              qkp[0:64, :])
                        nc.vector.tensor_copy(kt2[0:64, ts(tb, QB)],
                                              qkp[64:128, :])
                        nc.sync.dma_start(out=kt1[0:64, ts(tb, QB)],
                                          in_=z64[:, ts(tb, QB)])
                        nc.sync.dma_start(out=kt2[64:128, ts(tb, QB)],
                                          in_=z64[:, ts(tb, QB)])
                return run

            def unit_v(tb, st):
                def run():
                    xc = xc_cur[tb]
                    vp = aux.tile([128, HPC * D], F32, name="vp", tag="aux")
                    for e in range(NE):
                        nc.tensor.matmul(vp, xc[e][:, ts(st, 128)], wv_sb[e],
                                         start=(e == 0), stop=(e == NE - 1))
                    kt_idx = 4 * tb + st
                    for h in range(HPC):
                        nc.vector.tensor_copy(
                            vtok[h][:, kt_idx * 65: kt_idx * 65 + 64],
                            vp[:, ts(h, D)])
                    if st == 3:
                        nc.sync.dma_start(out=ot2[64:128, ts(tb, QB)],
                                          in_=z64[:, ts(tb, QB)])
                return run

            def qkv_units(tb):
                u = [unit_xdma(tb)]
                u += [unit_qk(tb, f) for f in range(3)]
                u += [unit_v(tb, st) for st in range(4)]
                return u

            # finalize J: stage the 3 heads' denominator rows side by side
            # on partition 0, one Ln + one Exp(scale=-1) on ACT for all
            # heads (1/den as exp(-ln(den))), then per head a ones-row
            # broadcast matmul and a DVE normalize multiply.
            otsb_of = {}
            recb_of = {}

            def unit_fin_stage(J):
                def run():
                    stg = osb.tile([1, HPC * QB], F32, name="stg", tag="stg",
                                   bufs=2)
                    for h in range(HPC):
                        nc.vector.tensor_copy(stg[:, ts(h, QB)],
                                              otsb_of[(h, J)][64:65, :])
                    lg = osb.tile([1, HPC * QB], F32, name="lg", tag="lg",
                                  bufs=2)
                    nc.scalar.activation(lg, stg,
                                         mybir.ActivationFunctionType.Ln)
                    recb = osb.tile([1, HPC * QB], BF16, name="recb",
                                    tag="recb", bufs=2)
                    nc.scalar.activation(recb, lg,
                                         mybir.ActivationFunctionType.Exp,
                                         scale=-1.0)
                    recb_of[J] = recb
                return run

            def unit_fin(h, J):
                def run():
                    otsb = otsb_of.pop((h, J))
                    bcp = aux.tile([64, QB], F32, name="bcp", tag="aux")
                    nc.tensor.matmul(bcp, on1_sb, recb_of[J][:, ts(h, QB)],
                                     start=True, stop=True)
                    dst = (ot01[0:64, ts(J, QB)], ot01[64:128, ts(J, QB)],
                           ot2[0:64, ts(J, QB)])[h]
                    nc.vector.tensor_mul(dst, otsb[0:64, :], bcp)
                return run

            def fin_units(J):
                return [unit_fin_stage(J)] + [unit_fin(h, J)
                                              for h in range(HPC)]

            # projection for token tile tt (128 tokens): 2 packed matmuls
            # per E/2 half, PSUM -> SBUF -> DRAM
            def unit_proj(tt):
                def run():
                    y_sb = osb.tile([128, E], F32, name="ysb", tag="ysb",
                                    bufs=2)
                    for eh in range(2):
                        pp = aux.tile([128, E // 2], F32, name="pp",
                                      tag="aux")
                        nc.tensor.matmul(pp, ot01[:, ts(tt, 128)],
                                         wp01_sb[:, ts(eh, E // 2)],
                                         start=True, stop=False)
                        nc.tensor.matmul(pp, ot2[:, ts(tt, 128)],
                                         wp2_sb[:, ts(eh, E // 2)],
                                         start=False, stop=True)
                        nc.vector.tensor_copy(y_sb[:, ts(eh, E // 2)], pp)
                    nc.sync.dma_start(out=y[ts(tt, 128), :], in_=y_sb)
                return run

            def proj_units(J):
                return [unit_proj(4 * J + t) for t in range(4)]

            # ----------------------------------------------------------
            # attention streams (scores^T layout)
            # ----------------------------------------------------------
            def c0_of(J, i):
                r = i - 4 * J
                return 0 if r < 0 else 128 * r

            def emit_s(h, J, g, u, sp):
                i = 2 * g + u
                c0 = c0_of(J, i)
                kts = kt_of(h)[:, ts(i, 128)]
                qts = qt_of(h)
                base = QB * u
                if i - 4 * J >= 0:
                    # diagonal tile: -30 mask + scores on the 128-wide
                    # diagonal block, plain scores on the rest
                    nc.tensor.matmul(sp[:, base + c0: base + c0 + 128],
                                     msk_sb, idn_sb, start=True, stop=False)
                    nc.tensor.matmul(sp[:, base + c0: base + c0 + 128],
                                     kts, qts[:, QB * J + c0: QB * J + c0 + 128],
                                     start=False, stop=True)
                    if c0 + 128 < QB:
                        nc.tensor.matmul(sp[:, base + c0 + 128: base + QB],
                                         kts,
                                         qts[:, QB * J + c0 + 128: QB * (J + 1)],
                                         start=True, stop=True)
                else:
                    nc.tensor.matmul(sp[:, base: base + QB],
                                     kts, qts[:, ts(J, QB)],
                                     start=True, stop=True)

            def emit_exp(h, J, g, sp):
                start = c0_of(J, 2 * g)
                if start < 0:
                    start = 0
                ex = asb.tile([128, 2 * QB], BF16, name="ex", tag="ex")
                nc.scalar.activation(ex[:, start:], sp[:, start:],
                                     mybir.ActivationFunctionType.Exp)
                return ex

            def emit_av(h, J, g, otp, ex):
                imax = 4 * J + 3
                for u in range(2):
                    i = 2 * g + u
                    c0 = c0_of(J, i)
                    nc.tensor.matmul(
                        otp[:, c0:QB],
                        vtok[h][:, i * 65:(i + 1) * 65],
                        ex[:, QB * u + c0: QB * (u + 1)],
                        start=(i == 0), stop=(i == imax))

            def drain_otp(h, J, otp):
                otsb = osb.tile([65, QB], F32, name="otsb", tag="otsb",
                                bufs=6)
                nc.vector.tensor_copy(otsb, otp)
                otsb_of[(h, J)] = otsb

            side = []

            def pop_side():
                if side:
                    side.pop(0)()

            def s_exp(h, J, g):
                sp = sps.tile([128, 2 * QB], F32, name="sp", tag="sp")
                for u in range(2):
                    emit_s(h, J, g, u, sp)
                return emit_exp(h, J, g, sp)

            # ----------------------------------------------------------
            # schedule: per q-block J, pass1 = heads 0+1 interleaved,
            # pass2 = head 2 + side work. At each segment boundary the
            # next segment's first scores+exp are emitted early so ACT
            # never runs dry waiting behind the current segment's AVs.
            # ----------------------------------------------------------
            for u_ in qkv_units(0):
                u_()

            PREFETCH_C = True  # pass1 -> pass2 handoff
            hand = None  # prefetched ex for the next segment's g=0
            for J in range(NQB):
                last = J == NQB - 1
                if J >= 1:
                    side.extend(fin_units(J - 1))
                    side.extend(proj_units(J - 1))
                if J + 1 < NQB:
                    side.extend(qkv_units(J + 1))
                ng = 2 * J + 2
                # pass 1: heads 0 and 1 as interleaved streams
                otpA = ops.tile([65, QB], F32, name="otpA", tag="otp")
                otpB = ops.tile([65, QB], F32, name="otpB", tag="otp")
                for g in range(ng):
                    if g == 0 and hand is not None:
                        exA = hand
                        hand = None
                    else:
                        exA = s_exp(0, J, g)
                    exB = s_exp(1, J, g)
                    if g == ng - 1 and PREFETCH_C:
                        handC = s_exp(2, J, 0)
                    if side and (not last or g % 3 == 0):
                        pop_side()
                        if len(side) > 8:
                            pop_side()
                    emit_av(0, J, g, otpA, exA)
                    emit_av(1, J, g, otpB, exB)
                drain_otp(0, J, otpA)
                drain_otp(1, J, otpB)
                # pass 2: head 2 alone, side work fills the PE
                otpC = ops.tile([65, QB], F32, name="otpC", tag="otp")
                for g in range(ng):
                    exC = handC if (g == 0 and PREFETCH_C) else s_exp(2, J, g)
                    if (g == ng - 1 and not last
                            and qk_done.get(J + 1, 0) == 3):
                        # safe only once block J+1's qt/kt drains are emitted
                        hand = s_exp(0, J + 1, 0)
                    pop_side()
                    pop_side()
                    emit_av(2, J, g, otpC, exC)
                drain_otp(2, J, otpC)

            side.extend(fin_units(NQB - 1))
            side.extend(proj_units(NQB - 1))
            while side:
                side.pop(0)()

    _split_multi_waits(nc)
    return nc


def _get_nc(with_bias):
    if with_bias not in _nc:
        _nc[with_bias] = _build_program(with_bias)
    return _nc[with_bias]


def _bf16(a):
    return np.ascontiguousarray(a.astype(ml_dtypes.bfloat16))


def kernel(x, W_attn, b_attn, W_proj, b_proj):
    x = np.asarray(x, dtype=np.float32)
    W_attn = np.asarray(W_attn, dtype=np.float32)
    b_attn = np.asarray(b_attn, dtype=np.float32)
    W_proj = np.asarray(W_proj, dtype=np.float32)
    b_proj = np.asarray(b_proj, dtype=np.float32)

    scale = 1.0 / np.sqrt(np.float32(D))

    # augmented x^T per batch: rows 0..767 = x[b]^T, row 768 = 1, rest 0
    xT_b = []
    for b in range(B):
        xa = np.zeros((EAUG, S), dtype=np.float32)
        xa[:E] = x[b].T
        xa[E] = 1.0
        xT_b.append(_bf16(xa))

    # -30 additive causal mask for the diagonal 128x128 block, stored
    # transposed (stationary operand against identity moving operand):
    # msk[q, k] = -30 if k > q else 0
    qi = np.arange(128)
    msk_np = _bf16(np.where(qi[None, :] > qi[:, None], -30.0, 0.0))
    idn_np = _bf16(np.eye(128, dtype=np.float32))
    on1_np = _bf16(np.ones((1, 64), dtype=np.float32))
    z64_np = _bf16(np.zeros((64, S), dtype=np.float32))

    in_maps = []
    for c in range(NCORES):
        b = c // 4
        heads = [HPC * (c % 4) + j for j in range(HPC)]
        # wqk: [EAUG, 384]; q cols pre-scaled by 1/sqrt(D) (bias row too).
        # Column order [q_h0|q_h1|q_h2|k_h0|k_h1|k_h2] so the f-groups are
        # [q_h0|q_h1], [q_h2|k_h0], [k_h1|k_h2].
        wqk = np.zeros((EAUG, 2 * HPC * D), dtype=np.float32)
        wv = np.zeros((EAUG, HPC * D), dtype=np.float32)
        col_of = {0: 0, 1: 1, 2: 2}          # q column slot per local head
        colk_of = {0: 3, 1: 4, 2: 5}         # k column slot per local head
        for j, h in enumerate(heads):
            wqk[:E, ts_(col_of[j])] = W_attn[:, h * D:(h + 1) * D] * scale
            wqk[E, ts_(col_of[j])] = b_attn[h * D:(h + 1) * D] * scale
            wqk[:E, ts_(colk_of[j])] = W_attn[:, E + h * D:E + (h + 1) * D]
            wqk[E, ts_(colk_of[j])] = b_attn[E + h * D:E + (h + 1) * D]
            wv[:E, ts_(j)] = W_attn[:, 2 * E + h * D:2 * E + (h + 1) * D]
            wv[E, ts_(j)] = b_attn[2 * E + h * D:2 * E + (h + 1) * D]
        # packed projection weights: [Wp_h0 ; Wp_h1] and [Wp_h2 ; 0]
        wp01 = np.concatenate(
            [W_proj[heads[0] * D:(heads[0] + 1) * D, :],
             W_proj[heads[1] * D:(heads[1] + 1) * D, :]], axis=0)
        wp2 = np.zeros((128, E), dtype=np.float32)
        wp2[0:64] = W_proj[heads[2] * D:(heads[2] + 1) * D, :]
        in_maps.append({
            "xT": xT_b[b],
            "wqk": _bf16(wqk),
            "wv": _bf16(wv),
            "wp01": _bf16(wp01),
            "wp2": _bf16(wp2),
            "msk": msk_np,
            "idn": idn_np,
            "on1": on1_np,
            "z64": z64_np,
        })

    with_bias = bool(np.any(b_attn != 0.0))
    nc = _get_nc(with_bias)
    global LAST_EXEC_NS
    if TRACE:
        _install_ntff_hook()
        res = run_bass_kernel_spmd(nc, in_maps, core_ids=list(range(NCORES)),
                                   trace=True)
        LAST_EXEC_NS = res.exec_time_ns
    else:
        res = run_bass_kernel_spmd(nc, in_maps, core_ids=list(range(NCORES)))

    y = np.zeros((B, S, E), dtype=np.float32)
    for c in range(NCORES):
        y[c // 4] += res.results[c]["y"]
    y += b_proj
    return y


def ts_(j):
    return slice(j * D, (j + 1) * D)


def _install_ntff_hook():
    """Register the axon NTFF profiling hook (dev/profiling only)."""
    import sys, types
    try:
        import antenv
        try:
            from antenv.axon_hooks import get_axon_ntff_profile_hook  # noqa
            return
        except ImportError:
            pass
        hooks_mod = types.ModuleType("antenv.axon_hooks")
        _hook = [None]
        hooks_mod.set_axon_ntff_profile_hook = lambda h: _hook.__setitem__(0, h)
        hooks_mod.get_axon_ntff_profile_hook = lambda: _hook[0]
        sys.modules["antenv.axon_hooks"] = hooks_mod
        antenv.axon_hooks = hooks_mod
        from trn_agent_boot.trn_boot import _ntff_profile_via_ctypes
        hooks_mod.set_axon_ntff_profile_hook(
            _ntff_profile_via_ctypes('/opt/axon/libaxon_pjrt.so'))
    except Exception:
        pass


# revision 50
# speedup vs baseline: 1.0686x; 1.0686x over previous
"""Causal self-attention (GPT-2 style) on 8 TRN2 NeuronCores.

Sharding: B=2 x H=12 -> 24 (batch, head) pairs; core c handles batch c//4
and heads [3*(c%4), 3*(c%4)+3). Each core computes QKV for its 3 heads,
causal attention (flash-style, scores^T layout), and a partial output
projection; the host sums the 4 per-batch partials and adds b_proj.

v2: single software-pipelined schedule. Attention for q-block J overlaps
with QKV for block J+1, the output projection for block J-1 and the
softmax finalize for J-1 (side-work units drained from a deque, one per
attention group). The causal diagonal mask is applied as a -30 additive
matmul into PSUM before exp (no DVE masking on the exp->AV critical
path). Softmax normalization avoids ACT entirely: denominator row is
drained with O^T to SBUF, reciprocal via DVE approx, broadcast across
partitions via gpsimd, multiply on DVE. ACT does exp only, which is the
global bottleneck engine; PE is kept dense (p-state!) by the side-work
interleave.

Self-contained: builds the Bass program on first call, runs via
run_bass_kernel_spmd on cores 0-7.
"""
import numpy as np
import ml_dtypes

import concourse.bass as bass
import concourse.mybir as mybir
import concourse.tile as tile
from concourse.bass import ts
from concourse.vector_clock import ScopedClock
from concourse.bass_utils import run_bass_kernel_spmd

# ---------------------------------------------------------------------------
# Workaround for the container's walrus build, which rejects any instruction
# carrying more than ONE sync-wait command ("Too many sync wait commands").
# 1) patch the TileContext tail drain to funnel its wait-set through
#    single-wait NOPs on SP; 2) post-pass that moves excess on_wait entries
#    from any instruction onto single-wait NOPs inserted before it on the
#    same engine (engine stalls on the NOPs, then issues the instruction —
#    semantics preserved).
# ---------------------------------------------------------------------------
_WAIT_LIMIT = 1


def _patched_drain_and_barrier(self, tick_clock, wait_clock):
    nc = self.nc
    carrier = nc.sync.nop()
    wait_clock.add_sem_waits(carrier.ins, ScopedClock({None: tick_clock.global_clock}))
    si = carrier.ins.sync_info
    waits = list(si.on_wait) if si and si.on_wait else []
    if len(waits) > _WAIT_LIMIT:
        si.on_wait = waits[:_WAIT_LIMIT]
        for w in waits[_WAIT_LIMIT:]:
            n2 = nc.sync.nop()
            s2 = n2.ins.sync_info
            if s2 is None:
                n2.ins.sync_info = mybir.SyncInfo(on_wait=[w], on_update=[])
            else:
                s2.on_wait = [w]
    nc.sync.drain()
    nc.all_engine_barrier()
    popped = nc._tile_sem_poison_stack.pop()
    assert popped is self._sem_poison
    nc.clear_and_free_semaphores(list(self.sems.allocated().values()))
    nc.all_engine_barrier()


tile.TileContext._drain_and_barrier = _patched_drain_and_barrier


def _split_multi_waits(nc):
    n_inserted = 0
    for fn in nc.m.functions:
        for blk in fn.blocks:
            new_list = []
            changed = False
            for inst in blk.instructions:
                si = getattr(inst, "sync_info", None)
                waits = list(si.on_wait) if (si is not None and si.on_wait) else []
                if len(waits) > _WAIT_LIMIT:
                    extra = waits[: len(waits) - _WAIT_LIMIT]
                    keep = waits[len(waits) - _WAIT_LIMIT:]
                    for w in extra:
                        nop = mybir.InstNoOp(
                            name=f"wsplit-{n_inserted}",
                            sync_info=mybir.SyncInfo(on_wait=[w], on_update=[]),
                            bass_nofuse=True,
                            engine=inst.engine,
                        )
                        new_list.append(nop)
                        n_inserted += 1
                    si.on_wait = keep
                    changed = True
                new_list.append(inst)
            if changed:
                blk.instructions = new_list
    return n_inserted


# ---------------------------------------------------------------------------
# Problem constants (hardcoded per contract).
# ---------------------------------------------------------------------------
B, S, E, H = 2, 4096, 768, 12
D = 64           # head dim
HPC = 3          # heads per core
EAUG = 832       # 768 + ones/bias row at 768, zero-padded to 6*128+64
NCORES = 8
BF16 = mybir.dt.bfloat16
F32 = mybir.dt.float32
F8 = mybir.dt.float8e4
DR = mybir.MatmulPerfMode.DoubleRow
QB = 512         # q-block width (one PSUM bank of fp32)
NQB = S // QB    # 8
NKT = S // 128   # 32 k-tiles

TRACE = False
LAST_EXEC_NS = None

_nc = {}


def _echunks(with_bias):
    # contraction chunks over the (augmented) feature dim
    ch = [(e * 128, 128) for e in range(6)]
    if with_bias:
        ch.append((768, 64))  # ones/bias row (+ zero padding)
    return ch


def _build_program(with_bias):
    nc = bass.Bass()
    xT = nc.dram_tensor("xT", [EAUG, S], BF16, kind="ExternalInput")
    xT8 = nc.dram_tensor("xT8", [EAUG, S], F8, kind="ExternalInput")
    wqk8 = nc.dram_tensor("wqk8", [EAUG, 2 * HPC * D], F8,
                          kind="ExternalInput")
    wv = nc.dram_tensor("wv", [EAUG, HPC * D], BF16, kind="ExternalInput")
    wp01 = nc.dram_tensor("wp01", [128, E], BF16, kind="ExternalInput")
    wp2 = nc.dram_tensor("wp2", [128, E], BF16, kind="ExternalInput")
    msk = nc.dram_tensor("msk", [128, 128], BF16, kind="ExternalInput")
    idn = nc.dram_tensor("idn", [128, 128], BF16, kind="ExternalInput")
    on1 = nc.dram_tensor("on1", [1, 64], BF16, kind="ExternalInput")
    y = nc.dram_tensor("y", [S, E], F32, kind="ExternalOutput")

    ech = _echunks(with_bias)
    NE = len(ech)

    with tile.TileContext(nc) as tc:
        with (
            tc.tile_pool(name="wpool", bufs=1) as wpool,
            tc.tile_pool(name="per", bufs=1) as per,
            tc.tile_pool(name="xch", bufs=12) as xch,
            tc.tile_pool(name="asb", bufs=6) as asb,
            tc.tile_pool(name="osb", bufs=1) as osb,
            tc.tile_pool(name="sps", bufs=2, space="PSUM") as sps,
            tc.tile_pool(name="ops", bufs=2, space="PSUM") as ops,
            tc.tile_pool(name="aux", bufs=2, space="PSUM") as aux,
        ):
            # --- weights + constants to SBUF (issued from the ACT queue so
            # the SP queue starts streaming x block 0 immediately) ---
            # wqk8: fp8 pairs for DoubleRow q/k projection (contraction
            # chunk 2p stacked with 2p+1 on the pair axis)
            NP = 3  # 128-row chunk pairs in E=768
            wqk8_sb = []
            for p in range(NP):
                t1 = wpool.tile([128, 2, 2 * HPC * D], F8, name=f"wqk8_{p}")
                nc.scalar.dma_start(out=t1[:, 0, :],
                                    in_=wqk8[256 * p:256 * p + 128, :])
                nc.scalar.dma_start(out=t1[:, 1, :],
                                    in_=wqk8[256 * p + 128:256 * p + 256, :])
                wqk8_sb.append(t1)
            if with_bias:
                wqk8b = wpool.tile([64, 2 * HPC * D], F8, name="wqk8b")
                nc.scalar.dma_start(out=wqk8b, in_=wqk8[768:832, :])
            wv_sb = []
            for e, (r0, rn) in enumerate(ech):
                t2 = wpool.tile([rn, HPC * D], BF16, name=f"wv{e}")
                nc.scalar.dma_start(out=t2, in_=wv[r0:r0 + rn, :])
                wv_sb.append(t2)
            wp01_sb = wpool.tile([128, E], BF16, name="wp01_sb")
            nc.scalar.dma_start(out=wp01_sb, in_=wp01[:, :])
            wp2_sb = wpool.tile([128, E], BF16, name="wp2_sb")
            nc.scalar.dma_start(out=wp2_sb, in_=wp2[:, :])
            msk_sb = wpool.tile([128, 128], BF16, name="msk_sb")
            nc.scalar.dma_start(out=msk_sb, in_=msk[:, :])
            idn_sb = wpool.tile([128, 128], BF16, name="idn_sb")
            nc.scalar.dma_start(out=idn_sb, in_=idn[:, :])
            on1_sb = wpool.tile([1, 64], BF16, name="on1_sb")
            nc.scalar.dma_start(out=on1_sb, in_=on1[:, :])

            # --- persistent intermediates ---
            # Feature-major Q^T/K^T. qt01 packs [q_h0 ; q_h1] on partitions
            # 0:64 / 64:128 (natural f-group drain). qt2 = [q_h2 ; k_h0]
            # (row filler is real data: multiplied by kt zero-pad).
            # kt[h] is per-head with ZEROS in the half where the paired q
            # tile carries the other head: kt0=[k_h0;0], kt1=[0;k_h1],
            # kt2=[k_h2;0]. All matmuls contract over full 128 partitions
            # (K=64 matmuls never un-throttle the PE clock gate).
            qt01 = per.tile([128, S], BF16, name="qt01")
            qt2 = per.tile([128, S], BF16, name="qt2")
            kt0 = per.tile([128, S], BF16, name="kt0")
            kt1 = per.tile([128, S], BF16, name="kt1")
            kt2 = per.tile([128, S], BF16, name="kt2")
            # zero pads via small per-block DVE memsets inside the QKV
            # units (big DMAs gate the first score matmuls; per-block pad
            # DMAs serialize the drains behind DMA-completion semaphores)

            def qt_of(h):
                return (qt01, qt01, qt2)[h]

            def kt_of(h):
                return (kt0, kt1, kt2)[h]

            # vtok[h]: token-major V with a ones column per k-tile (bf16:
            # fp8 V puts ~1.8% RMS noise straight onto the output and
            # fails the accuracy gate)
            vtok = [per.tile([128, 65 * NKT], BF16, name=f"vtok{h}")
                    for h in range(HPC)]
            for h in range(HPC):
                nc.vector.memset(vtok[h], 1.0)
            # normalized O^T packed for projection: ot01=[o_h0;o_h1],
            # ot2=[o_h2;0]
            ot01 = per.tile([128, S], BF16, name="ot01")
            ot2 = per.tile([128, S], BF16, name="ot2")

            # ----------------------------------------------------------
            # work units
            # ----------------------------------------------------------
            def unit_xdma(tb):
                def run():
                    xc8 = []
                    for p in range(NP):
                        t8 = xch.tile([128, 2, QB], F8, name=f"xc8_{p}",
                                      tag="xc8", bufs=6)
                        nc.sync.dma_start(
                            out=t8[:, 0, :],
                            in_=xT8[256 * p:256 * p + 128, ts(tb, QB)])
                        nc.sync.dma_start(
                            out=t8[:, 1, :],
                            in_=xT8[256 * p + 128:256 * p + 256, ts(tb, QB)])
                        xc8.append(t8)
                    if with_bias:
                        t8 = xch.tile([64, QB], F8, name="xc8b", tag="xc8b",
                                      bufs=2)
                        nc.sync.dma_start(out=t8,
                                          in_=xT8[768:832, ts(tb, QB)])
                        xc8.append(t8)
                    xc = []
                    for e, (r0, rn) in enumerate(ech):
                        t = xch.tile([rn, QB], BF16, name=f"xc{e}", tag="xc")
                        nc.sync.dma_start(out=t, in_=xT[r0:r0 + rn, ts(tb, QB)])
                        xc.append(t)
                    xc_cur[tb] = (xc, xc8)
                return run

            xc_cur = {}

            qk_done = {}

            def unit_qk(tb, f):
                def run():
                    qk_done[tb] = qk_done.get(tb, 0) + 1
                    _, xc8 = xc_cur[tb]
                    qkp = aux.tile([128, QB], F32, name="qkp", tag="aux")
                    for p in range(NP):
                        nc.tensor.matmul(qkp,
                                         wqk8_sb[p][:, :, ts(f, 128)],
                                         xc8[p],
                                         start=(p == 0),
                                         stop=(p == NP - 1 and not with_bias),
                                         perf_mode=DR)
                    if with_bias:
                        nc.tensor.matmul(qkp, wqk8b[:, ts(f, 128)], xc8[NP],
                                         start=False, stop=True)
                    if f == 0:      # [q_h0 ; q_h1]
                        nc.vector.tensor_copy(qt01[:, ts(tb, QB)], qkp)
                    elif f == 1:    # [q_h2 ; k_h0]
                        nc.vector.tensor_copy(qt2[:, ts(tb, QB)], qkp)
                        nc.vector.tensor_copy(kt0[0:64, ts(tb, QB)],
                                              qkp[64:128, :])
                        nc.vector.memset(kt0[64:128, ts(tb, QB)], 0.0)
                    else:           # [k_h1 ; k_h2]
                        nc.vector.tensor_copy(kt1[64:128, ts(tb, QB)],
                                              qkp[0:64, :])
                        nc.vector.tensor_copy(kt2[0:64, ts(tb, QB)],
                                              qkp[64:128, :])
                        nc.vector.memset(kt1[0:64, ts(tb, QB)], 0.0)
                        nc.vector.memset(kt2[64:128, ts(tb, QB)], 0.0)
                return run

            def unit_v(tb, st):
                def run():
                    xc, _ = xc_cur[tb]
                    vp = aux.tile([128, HPC * D], F32, name="vp", tag="aux")
                    for e in range(NE):
                        nc.tensor.matmul(vp, xc[e][:, ts(st, 128)], wv_sb[e],
                                         start=(e == 0), stop=(e == NE - 1))
                    kt_idx = 4 * tb + st
                    for h in range(HPC):
                        nc.vector.tensor_copy(
                            vtok[h][:, kt_idx * 65: kt_idx * 65 + 64],
                            vp[:, ts(h, D)])
                    if st == 3:
                        nc.vector.memset(ot2[64:128, ts(tb, QB)], 0.0)
                return run

            def qkv_units(tb):
                u = [unit_xdma(tb)]
                u += [unit_qk(tb, f) for f in range(3)]
                u += [unit_v(tb, st) for st in range(4)]
                return u

            # finalize J: stage the 3 heads' denominator rows side by side
            # on partition 0, one Ln + one Exp(scale=-1) on ACT for all
            # heads (1/den as exp(-ln(den))), then per head a ones-row
            # broadcast matmul and a DVE normalize multiply.
            otsb_of = {}
            recb_of = {}

            def unit_fin_stage(J):
                def run():
                    stg = osb.tile([1, HPC * QB], F32, name="stg", tag="stg",
                                   bufs=2)
                    for h in range(HPC):
                        nc.vector.tensor_copy(stg[:, ts(h, QB)],
                                              otsb_of[(h, J)][64:65, :])
                    lg = osb.tile([1, HPC * QB], F32, name="lg", tag="lg",
                                  bufs=2)
                    nc.scalar.activation(lg, stg,
                                         mybir.ActivationFunctionType.Ln)
                    recb = osb.tile([1, HPC * QB], BF16, name="recb",
                                    tag="recb", bufs=2)
                    nc.scalar.activation(recb, lg,
                                         mybir.ActivationFunctionType.Exp,
                                         scale=-1.0)
                    recb_of[J] = recb
                return run

            def unit_fin(h, J):
                def run():
                    otsb = otsb_of.pop((h, J))
                    bcp = aux.tile([64, QB], F32, name="bcp", tag="aux")
                    nc.tensor.matmul(bcp, on1_sb, recb_of[J][:, ts(h, QB)],
                                     start=True, stop=True)
                    dst = (ot01[0:64, ts(J, QB)], ot01[64:128, ts(J, QB)],
                           ot2[0:64, ts(J, QB)])[h]
                    nc.vector.tensor_mul(dst, otsb[0:64, :], bcp)
                return run

            def fin_units(J):
                return [unit_fin_stage(J)] + [unit_fin(h, J)
                                              for h in range(HPC)]

            # projection for token tile tt (128 tokens): 2 packed matmuls
            # per E/2 half, PSUM -> SBUF -> DRAM
            def unit_proj(tt):
                def run():
                    y_sb = osb.tile([128, E], F32, name="ysb", tag="ysb",
                                    bufs=2)
                    for eh in range(2):
                        pp = aux.tile([128, E // 2], F32, name="pp",
                                      tag="aux")
                        nc.tensor.matmul(pp, ot01[:, ts(tt, 128)],
                                         wp01_sb[:, ts(eh, E // 2)],
                                         start=True, stop=False)
                        nc.tensor.matmul(pp, ot2[:, ts(tt, 128)],
                                         wp2_sb[:, ts(eh, E // 2)],
                                         start=False, stop=True)
                        nc.vector.tensor_copy(y_sb[:, ts(eh, E // 2)], pp)
                    nc.sync.dma_start(out=y[ts(tt, 128), :], in_=y_sb)
                return run

            def proj_units(J):
                return [unit_proj(4 * J + t) for t in range(4)]

            # ----------------------------------------------------------
            # attention streams (scores^T layout)
            # ----------------------------------------------------------
            def c0_of(J, i):
                r = i - 4 * J
                return 0 if r < 0 else 128 * r

            def emit_s(h, J, g, u, sp):
                i = 2 * g + u
                c0 = c0_of(J, i)
                kts = kt_of(h)[:, ts(i, 128)]
                qts = qt_of(h)
                if i - 4 * J >= 0:
                    # diagonal tile: masked scores on the 128-wide
                    # diagonal block, plain scores on the rest
                    nc.tensor.matmul(sp[:, u, c0:c0 + 128],
                                     msk_sb, idn_sb, start=True, stop=False)
                    nc.tensor.matmul(sp[:, u, c0:c0 + 128],
                                     kts, qts[:, QB * J + c0: QB * J + c0 + 128],
                                     start=False, stop=True)
                    if c0 + 128 < QB:
                        nc.tensor.matmul(sp[:, u, c0 + 128: QB],
                                         kts,
                                         qts[:, QB * J + c0 + 128: QB * (J + 1)],
                                         start=True, stop=True)
                else:
                    nc.tensor.matmul(sp[:, u, 0:QB],
                                     kts, qts[:, ts(J, QB)],
                                     start=True, stop=True)

            def emit_exp(h, J, g, sp):
                start = c0_of(J, 2 * g)
                if start < 0:
                    start = 0
                ex = asb.tile([128, 2, QB], BF16, name="ex", tag="ex")
                # scores arrive x512 (fp8 weight rescale); fold 1/512 into
                # the activation's free scale
                nc.scalar.activation(ex[:, :, start:], sp[:, :, start:],
                                     mybir.ActivationFunctionType.Exp,
                                     scale=1.0 / 512.0)
                return ex

            def emit_av(h, J, g, otp, ex):
                imax = 4 * J + 3
                for u in range(2):
                    i = 2 * g + u
                    c0 = c0_of(J, i)
                    nc.tensor.matmul(
                        otp[:, c0:QB],
                        vtok[h][:, i * 65:(i + 1) * 65],
                        ex[:, u, c0:],
                        start=(i == 0), stop=(i == imax))

            OTP_P = 65

            def drain_otp(h, J, otp):
                otsb = osb.tile([65, QB], F32, name="otsb", tag="otsb",
                                bufs=6)
                nc.vector.tensor_copy(otsb, otp[0:65, :])
                otsb_of[(h, J)] = otsb

            side = []

            def pop_side():
                if side:
                    side.pop(0)()

            def s_exp(h, J, g):
                sp = sps.tile([128, 2, QB], F32, name="sp", tag="sp")
                for u in range(2):
                    emit_s(h, J, g, u, sp)
                return emit_exp(h, J, g, sp)

            # ----------------------------------------------------------
            # schedule: per q-block J, pass1 = heads 0+1 interleaved,
            # pass2 = head 2 + side work. At each segment boundary the
            # next segment's first scores+exp are emitted early so ACT
            # never runs dry waiting behind the current segment's AVs.
            # ----------------------------------------------------------
            for u_ in qkv_units(0):
                u_()

            PREFETCH_C = True  # pass1 -> pass2 handoff
            hand = None  # prefetched ex for the next segment's g=0
            for J in range(NQB):
                last = J == NQB - 1
                if J >= 1:
                    side.extend(fin_units(J - 1))
                    side.extend(proj_units(J - 1))
                if J + 1 < NQB:
                    side.extend(qkv_units(J + 1))
                ng = 2 * J + 2
                # pass 1: heads 0 and 1 as interleaved streams
                otpA = ops.tile([OTP_P, QB], F32, name="otpA", tag="otp")
                otpB = ops.tile([OTP_P, QB], F32, name="otpB", tag="otp")
                for g in range(ng):
                    if g == 0 and hand is not None:
                        exA = hand
                        hand = None
                    else:
                        exA = s_exp(0, J, g)
                    exB = s_exp(1, J, g)
                    if g == ng - 1 and PREFETCH_C:
                        handC = s_exp(2, J, 0)
                    if side and (not last or g % 3 == 0):
                        pop_side()
                        if len(side) > 8:
                            pop_side()
                    emit_av(0, J, g, otpA, exA)
                    emit_av(1, J, g, otpB, exB)
                drain_otp(0, J, otpA)
                drain_otp(1, J, otpB)
                # pass 2: head 2 alone, side work fills the PE
                otpC = ops.tile([OTP_P, QB], F32, name="otpC", tag="otp")
                for g in range(ng):
                    exC = handC if (g == 0 and PREFETCH_C) else s_exp(2, J, g)
                    if (g == ng - 1 and not last
                            and qk_done.get(J + 1, 0) == 3):
                        # safe only once block J+1's qt/kt drains are emitted
                        hand = s_exp(0, J + 1, 0)
                    pop_side()
                    pop_side()
                    emit_av(2, J, g, otpC, exC)
                drain_otp(2, J, otpC)

            side.extend(fin_units(NQB - 1))
            side.extend(proj_units(NQB - 1))
            while side:
                side.pop(0)()

    _split_multi_waits(nc)
    return nc


def _get_nc(with_bias):
    if with_bias not in _nc:
        _nc[with_bias] = _build_program(with_bias)
    return _nc[with_bias]


def _bf16(a):
    return np.ascontiguousarray(a.astype(ml_dtypes.bfloat16))


def _f8(a):
    a = np.clip(a, -240.0, 240.0)
    return np.ascontiguousarray(a.astype(ml_dtypes.float8_e4m3fn))


def kernel(x, W_attn, b_attn, W_proj, b_proj):
    x = np.asarray(x, dtype=np.float32)
    W_attn = np.asarray(W_attn, dtype=np.float32)
    b_attn = np.asarray(b_attn, dtype=np.float32)
    W_proj = np.asarray(W_proj, dtype=np.float32)
    b_proj = np.asarray(b_proj, dtype=np.float32)

    # q/k weights are stored x8 in fp8 (typical |W_attn| ~0.016 sits at
    # fp8e4m3's subnormal floor); raw scores come out x64, and with the
    # 1/sqrt(d)=0.125 softmax scale folded in the exp reads them x512.
    QKS = 8.0

    # augmented x^T per batch: rows 0..767 = x[b]^T, row 768 = 1, rest 0
    xT_b, xT8_b = [], []
    for b in range(B):
        xa = np.zeros((EAUG, S), dtype=np.float32)
        xa[:E] = x[b].T
        xa[E] = 1.0
        xT_b.append(_bf16(xa))
        xT8_b.append(_f8(xa))

    # causal mask for the diagonal 128x128 block, stored transposed
    # (stationary operand against identity moving operand):
    # msk[q, k] = -30*512 if k > q else 0 (scores are x512)
    qi = np.arange(128)
    msk_np = _bf16(np.where(qi[None, :] > qi[:, None], -30.0 * 512.0, 0.0))
    idn_np = _bf16(np.eye(128, dtype=np.float32))
    on1_np = _bf16(np.ones((1, 64), dtype=np.float32))

    in_maps = []
    for c in range(NCORES):
        b = c // 4
        heads = [HPC * (c % 4) + j for j in range(HPC)]
        # wqk: [EAUG, 384]; q cols pre-scaled by 1/sqrt(D) (bias row too).
        # Column order [q_h0|q_h1|q_h2|k_h0|k_h1|k_h2] so the f-groups are
        # [q_h0|q_h1], [q_h2|k_h0], [k_h1|k_h2].
        wqk = np.zeros((EAUG, 2 * HPC * D), dtype=np.float32)
        wv = np.zeros((EAUG, HPC * D), dtype=np.float32)
        col_of = {0: 0, 1: 1, 2: 2}          # q column slot per local head
        colk_of = {0: 3, 1: 4, 2: 5}         # k column slot per local head
        for j, h in enumerate(heads):
            wqk[:E, ts_(col_of[j])] = W_attn[:, h * D:(h + 1) * D] * QKS
            wqk[E, ts_(col_of[j])] = b_attn[h * D:(h + 1) * D] * QKS
            wqk[:E, ts_(colk_of[j])] = \
                W_attn[:, E + h * D:E + (h + 1) * D] * QKS
            wqk[E, ts_(colk_of[j])] = b_attn[E + h * D:E + (h + 1) * D] * QKS
            wv[:E, ts_(j)] = W_attn[:, 2 * E + h * D:2 * E + (h + 1) * D]
            wv[E, ts_(j)] = b_attn[2 * E + h * D:2 * E + (h + 1) * D]
        # packed projection weights: [Wp_h0 ; Wp_h1] and [Wp_h2 ; 0]
        wp01 = np.concatenate(
            [W_proj[heads[0] * D:(heads[0] + 1) * D, :],
             W_proj[heads[1] * D:(heads[1] + 1) * D, :]], axis=0)
        wp2 = np.zeros((128, E), dtype=np.float32)
        wp2[0:64] = W_proj[heads[2] * D:(heads[2] + 1) * D, :]
        in_maps.append({
            "xT": xT_b[b],
            "xT8": xT8_b[b],
            "wqk8": _f8(wqk),
            "wv": _bf16(wv),
            "wp01": _bf16(wp01),
            "wp2": _bf16(wp2),
            "msk": msk_np,
            "idn": idn_np,
            "on1": on1_np,
        })

    with_bias = bool(np.any(b_attn != 0.0))
    nc = _get_nc(with_bias)
    global LAST_EXEC_NS
    if TRACE:
        _install_ntff_hook()
        res = run_bass_kernel_spmd(nc, in_maps, core_ids=list(range(NCORES)),
                                   trace=True)
        LAST_EXEC_NS = res.exec_time_ns
    else:
        res = run_bass_kernel_spmd(nc, in_maps, core_ids=list(range(NCORES)))

    y = np.zeros((B, S, E), dtype=np.float32)
    for c in range(NCORES):
        y[c // 4] += res.results[c]["y"]
    y += b_proj
    return y


def ts_(j):
    return slice(j * D, (j + 1) * D)


def _install_ntff_hook():
    """Register the axon NTFF profiling hook (dev/profiling only)."""
    import sys, types
    try:
        import antenv
        try:
            from antenv.axon_hooks import get_axon_ntff_profile_hook  # noqa
            return
        except ImportError:
            pass
        hooks_mod = types.ModuleType("antenv.axon_hooks")
        _hook = [None]
        hooks_mod.set_axon_ntff_profile_hook = lambda h: _hook.__setitem__(0, h)
        hooks_mod.get_axon_ntff_profile_hook = lambda: _hook[0]
        sys.modules["antenv.axon_hooks"] = hooks_mod
        antenv.axon_hooks = hooks_mod
        from trn_agent_boot.trn_boot import _ntff_profile_via_ctypes
        hooks_mod.set_axon_ntff_profile_hook(
            _ntff_profile_via_ctypes('/opt/axon/libaxon_pjrt.so'))
    except Exception:
        pass
